# revision 11
# baseline (speedup 1.0000x reference)
"""Chamfer distance (squared-L2) kernel for Trainium2 NeuronCores (axon).

Problem: xyz1 (4, 8192, 3) f32, xyz2 (4, 8192, 3) f32.
  d[b,n,m] = ||p_n - q_m||^2 ; out = mean_n(min_m d) + mean_m(min_n d)  (scalar f32)

The warm-path cost in this environment is one axon tunnel round trip
(~60-90 ms) plus transfer bytes (~10-25 ms/MB); on-device compute (~1.5 ms)
is invisible inside that window.  The design therefore minimizes round
trips and bytes:

  - ONE round trip: a single cached jitted shard_map call with numpy args
    and a SINGLE output array fetched by a single np.asarray -- upload,
    execute and fetch all pipeline into one tunnel round trip.
  - Minimal upload: data-parallel over B on 4 cores, one full batch per
    core, so every point is uploaded exactly once (786 KB total: per core
    a = -2*xyz1[b]^T and q = xyz2[b]^T, both (3, 8192) f32).  Splitting N
    further across 8 cores would duplicate xyz2 (+50% bytes) for ~0.8 ms
    of hidden device time -- a strictly worse trade here.
  - Minimal fetch: per-core column-mins are COMPLETE (the core sees all
    rows of its batch), and both reductions are finished on device down
    to two scalars per core ([sum row-mins, sum col-mins], (1,2) f32) --
    the fetch is 32 bytes total.

Device kernel (per core, batch b):
  - Augmented K=9 fp32 matmul emits complete squared-distance tiles:
      lhsT = [a_x a_y a_z | p_x^2 p_y^2 p_z^2 | 1 1 1]      (9, 8192)
      rhs  = [q_x q_y q_z | 1     1     1     | q_x^2 q_y^2 q_z^2]
    so lhsT.T @ rhs = ||p||^2 + ||q||^2 - 2 p.q exactly in fp32.  The
    operand rows are built on device: squares via one ACT Square per side
    (scale=0.5 turns a=-2p into p^2), ones via memset; compute engines
    need 32-aligned SBUF partition bases, so these go through a
    partition-0 scratch tile and are DMA'd (no alignment constraint) into
    rows 3..8.  fp32 matmul streams at 4 cyc/col -> ~1.5 ms PE time.
  - ScalarE copies PSUM distance tiles to SBUF narrowed to bf16
    (round-to-nearest noise on the mins averages out over 32k rows/cols).
  - VectorE tensor_reduce(min) per 128-row tile -> row-mins; a bf16
    running accumulator updated with tensor_tensor(min) -> column-mins.
  - PE transposes the (128, 8192) column-min accumulator in 128x128
    blocks; VectorE segmented min-reduces produce per-column mins.
  - A final ones-matmul folds the partition axis (bf16 mins summed
    exactly in f32 PSUM) and a segmented add-reduce emits the two
    per-core scalars.

Host (~1 ms): transpose/scale views into the concat upload buffer; sum
the 8 fetched scalars in f64.
"""

import os
import numpy as np
import ml_dtypes

B = 4
N = 8192
M = 8192
CORES = 4                # data-parallel over B: one batch per core
P = 128                  # partitions
NT = N // P              # 64 n-tiles
CHUNK = 2048             # columns per PSUM macro-tile
NCH = M // CHUNK         # 4 chunks
MMF = 512                # matmul free dim (one PSUM bank of fp32)
KAUG = 9                 # augmented contraction size (fp32 rows)
NBLK = M // P            # 64 column blocks of 128 for the final fold
OUTW = NT + NBLK         # 128 output columns: [rowmin | colmin]

BF16 = ml_dtypes.bfloat16

_NC_CACHE = {}


def _build_nc():
    import concourse.bass as bass
    import concourse.mybir as mybir
    import concourse.tile as tile
    import concourse.bacc as bacc
    from concourse.masks import make_identity
    from contextlib import ExitStack

    f32 = mybir.dt.float32
    bf16 = mybir.dt.bfloat16
    MIN = mybir.AluOpType.min
    AXX = mybir.AxisListType.X
    SQUARE = mybir.ActivationFunctionType.Square

    ADD = mybir.AluOpType.add

    nc = bacc.Bacc(trn_type="TRN2")
    # single input tensor: rows 0:3 = a = -2*xyz1[b]^T, rows 3:6 = xyz2[b]^T
    pts_d = nc.dram_tensor("pts", (6, N), f32, kind="ExternalInput").ap()
    # single tiny output: [sum of row-mins, sum of col-mins]
    out_d = nc.dram_tensor("out", (1, 2), f32, kind="ExternalOutput").ap()

    with tile.TileContext(nc) as tc, ExitStack() as ctx:
        consts = ctx.enter_context(tc.tile_pool(name="consts", bufs=1))
        accp = ctx.enter_context(tc.tile_pool(name="accp", bufs=1))
        psum = ctx.enter_context(tc.tile_pool(name="psum", bufs=2, space="PSUM"))
        dsb = ctx.enter_context(tc.tile_pool(name="dsb", bufs=3))
        outp = ctx.enter_context(tc.tile_pool(name="outp", bufs=1))

        # augmented matmul operands, built from the raw points via one
        # shared partition-0 scratch tile (see module docstring)
        aug1 = consts.tile([KAUG, N], f32)
        aug2 = consts.tile([KAUG, M], f32)
        scr = consts.tile([3, M], f32)
        nc.sync.dma_start(out=aug1[0:3, :], in_=pts_d[0:3, :])
        nc.gpsimd.dma_start(out=aug2[0:3, :], in_=pts_d[3:6, :])
        nc.vector.memset(scr, 1.0)
        nc.sync.dma_start(out=aug1[6:9, :], in_=scr)
        nc.sync.dma_start(out=aug2[3:6, :], in_=scr)
        # p^2 rows: a = -2p, so (0.5*a)^2 = p^2
        nc.scalar.activation(out=scr, in_=aug1[0:3, :], func=SQUARE, scale=0.5)
        nc.sync.dma_start(out=aug1[3:6, :], in_=scr)
        nc.scalar.activation(out=scr, in_=aug2[0:3, :], func=SQUARE)
        nc.sync.dma_start(out=aug2[6:9, :], in_=scr)

        ident = consts.tile([P, P], bf16)
        make_identity(nc, ident)

        # column-min accumulator, bf16 (DVE tensor_tensor min runs at
        # 2x_1P for bf16 SBUF operands)
        acc = accp.tile([P, M], bf16)

        rmall = outp.tile([P, NT], f32)
        out_sb = outp.tile([P, OUTW], bf16)

        repeat = int(os.environ.get("CHAMFER_REPEAT", "1"))
        for rep in range(repeat):
          for t in range(NT):
            d = dsb.tile([P, M], bf16, tag="d")
            for c in range(NCH):
                ps = psum.tile([P, CHUNK], f32, tag="ps")
                for j in range(CHUNK // MMF):
                    col = c * CHUNK + j * MMF
                    nc.tensor.matmul(
                        ps[:, j * MMF:(j + 1) * MMF],
                        aug1[:, t * P:(t + 1) * P],
                        aug2[:, col:col + MMF],
                        start=True,
                        stop=True,
                    )
                # ACT copies + narrows to bf16 (min results only need bf16:
                # round-to-nearest noise averages out over 32k rows/cols)
                nc.scalar.copy(out=d[:, c * CHUNK:(c + 1) * CHUNK], in_=ps)

            nc.vector.tensor_reduce(
                out=rmall[:, t:t + 1], in_=d, axis=AXX, op=MIN
            )
            if t == 0 and rep == 0:
                nc.vector.tensor_copy(out=acc, in_=d)
            else:
                nc.vector.tensor_tensor(out=acc, in0=d, in1=acc, op=MIN)

        # row-min columns of the output (bf16 narrowing is lossless: the
        # f32 values are mins of bf16 numbers)
        nc.scalar.copy(out=out_sb[:, :NT], in_=rmall)

        # fold the column-min accumulator over the partition axis:
        # PE-transpose 128x128 bf16 blocks into PSUM, then segmented
        # min-reduce straight into the output tile.
        TGRP = 8
        for g in range(NBLK // TGRP):
            psT = psum.tile([P, TGRP * P], bf16, tag="ps")
            for j in range(TGRP):
                k = g * TGRP + j
                nc.tensor.transpose(
                    psT[:, j * P:(j + 1) * P], acc[:, k * P:(k + 1) * P], ident
                )
            seg = psT.rearrange("p (j x) -> p j x", x=P)
            nc.vector.tensor_reduce(
                out=out_sb[:, NT + g * TGRP:NT + (g + 1) * TGRP],
                in_=seg, axis=AXX, op=MIN,
            )

        # fold out_sb ([rowmin | colmin], (128, 128) bf16) to two scalars:
        # ones-matmul sums the partition axis into PSUM (exact: bf16 values
        # accumulated in f32), then a segmented add-reduce sums each half.
        ones128 = consts.tile([P, 1], bf16)
        nc.vector.memset(ones128, 1.0)
        psS = psum.tile([1, OUTW], f32, tag="ps")
        nc.tensor.matmul(psS, ones128, out_sb, start=True, stop=True)
        segS = psS.rearrange("p (j x) -> p j x", x=NT)
        out_fin = outp.tile([1, 2], f32)
        nc.vector.tensor_reduce(out=out_fin, in_=segS, axis=AXX, op=ADD)
        nc.sync.dma_start(out=out_d, in_=out_fin)
    nc.compile()
    return nc


def _get_runner():
    """Build (once) a cached jitted SPMD executor for the bass program.

    Mirrors concourse.bass2jax.run_bass_via_pjrt's multi-core path, but
    caches the jitted callable so repeat kernel() calls skip
    retrace/recompile, and fetches the single global output with one
    np.asarray call -- upload, execute and fetch then pipeline into a
    single tunnel round trip.
    """
    if "runner" in _NC_CACHE:
        return _NC_CACHE["runner"]

    import jax
    import concourse.mybir as mybir
    from jax.experimental.shard_map import shard_map
    from jax.sharding import Mesh, PartitionSpec
    from concourse.bass2jax import (
        install_neuronx_cc_hook,
        partition_id_tensor,
        _bass_exec_p,
    )

    install_neuronx_cc_hook()
    nc = _build_nc()

    in_names, out_names, out_avals, zero_outs = [], [], [], []
    partition_name = nc.partition_id_tensor.name if nc.partition_id_tensor else None
    for alloc in nc.m.functions[0].allocations:
        if not isinstance(alloc, mybir.MemoryLocationSet):
            continue
        name = alloc.memorylocations[0].name
        if alloc.kind == "ExternalInput":
            if name != partition_name:
                in_names.append(name)
        elif alloc.kind == "ExternalOutput":
            shape = tuple(alloc.tensor_shape)
            dtype = mybir.dt.np(alloc.dtype)
            out_names.append(name)
            out_avals.append(jax.core.ShapedArray(shape, dtype))
            zero_outs.append(np.zeros((CORES * shape[0], *shape[1:]), dtype))
    n_params = len(in_names)
    all_in_names = list(in_names) + list(out_names)
    if partition_name is not None:
        all_in_names.append(partition_name)
    donate = tuple(range(n_params, n_params + len(out_names)))

    def _body(*args):
        operands = list(args)
        if partition_name is not None:
            operands.append(partition_id_tensor())
        outs = _bass_exec_p.bind(
            *operands,
            out_avals=tuple(out_avals),
            in_names=tuple(all_in_names),
            out_names=tuple(out_names),
            lowering_input_output_aliases=(),
            sim_require_finite=True,
            sim_require_nnan=True,
            nc=nc,
        )
        return tuple(outs)

    devices = jax.devices()[:CORES]
    mesh = Mesh(np.asarray(devices), ("core",))
    in_specs = (PartitionSpec("core"),) * (n_params + len(out_names))
    out_specs = (PartitionSpec("core"),) * len(out_names)
    sharded = jax.jit(
        shard_map(
            _body, mesh=mesh, in_specs=in_specs, out_specs=out_specs, check_rep=False
        ),
        donate_argnums=donate,
        keep_unused=True,
    )

    def run(pts_cat):
        out = sharded(pts_cat, *zero_outs)
        # single fetch of the single global output: one round trip total
        return np.asarray(out[0])

    # the first couple of executions of a fresh jitted callable are ~2x
    # slower (dispatch fast-path + donation warm-up); absorb them into the
    # cold path so every caller-visible call runs at steady state
    dummy = np.zeros((CORES * 6, N), np.float32)
    for _ in range(2):
        run(dummy)

    _NC_CACHE["runner"] = run
    return run


def kernel(xyz1, xyz2):
    xyz1 = np.asarray(xyz1)
    xyz2 = np.asarray(xyz2)

    # per-core (= per-batch) input, concatenated along axis 0 for shard_map:
    # rows 0:3 = -2*xyz1[b]^T, rows 3:6 = xyz2[b]^T
    pts_cat = np.empty((CORES * 6, N), np.float32)
    for b in range(CORES):
        np.multiply(xyz1[b].T, -2.0, out=pts_cat[b * 6:b * 6 + 3])
        pts_cat[b * 6 + 3:b * 6 + 6] = xyz2[b].T

    out = _get_runner()(pts_cat)  # (CORES*1, 2) f32

    out64 = out.astype(np.float64)
    val = out64[:, 0].sum() / (B * N) + out64[:, 1].sum() / (B * M)
    return np.asarray(val, dtype=np.float32)


# revision 17
# speedup vs baseline: 1.0285x; 1.0285x over previous
"""Chamfer distance (squared-L2) kernel for Trainium2 NeuronCores (axon).

Problem: xyz1 (4, 8192, 3) f32, xyz2 (4, 8192, 3) f32.
  d[b,n,m] = ||p_n - q_m||^2 ; out = mean_n(min_m d) + mean_m(min_n d)  (scalar f32)

The warm-path cost in this environment is one axon tunnel round trip
(~60-90 ms) plus transfer bytes (~10-25 ms/MB); on-device compute (~1.5 ms)
is invisible inside that window.  The design therefore minimizes round
trips and bytes:

  - ONE round trip: a single cached jitted shard_map call with numpy args
    and a SINGLE output array fetched by a single np.asarray -- upload,
    execute and fetch all pipeline into one tunnel round trip.
  - Minimal upload: data-parallel over B on 4 cores, one full batch per
    core, so every point is uploaded exactly once (786 KB total: per core
    a = -2*xyz1[b]^T and q = xyz2[b]^T, both (3, 8192) f32).  Splitting N
    further across 8 cores would duplicate xyz2 (+50% bytes) for ~0.8 ms
    of hidden device time -- a strictly worse trade here.
  - Minimal fetch: per-core column-mins are COMPLETE (the core sees all
    rows of its batch), and both reductions are finished on device down
    to two scalars per core ([sum row-mins, sum col-mins], (1,2) f32) --
    the fetch is 32 bytes total.

Device kernel (per core, batch b):
  - Augmented K=9 fp32 matmul emits complete squared-distance tiles:
      lhsT = [a_x a_y a_z | p_x^2 p_y^2 p_z^2 | 1 1 1]      (9, 8192)
      rhs  = [q_x q_y q_z | 1     1     1     | q_x^2 q_y^2 q_z^2]
    so lhsT.T @ rhs = ||p||^2 + ||q||^2 - 2 p.q exactly in fp32.  The
    operand rows are built on device: squares via one ACT Square per side
    (scale=0.5 turns a=-2p into p^2), ones via memset; compute engines
    need 32-aligned SBUF partition bases, so these go through a
    partition-0 scratch tile and are DMA'd (no alignment constraint) into
    rows 3..8.  fp32 matmul streams at 4 cyc/col -> ~1.5 ms PE time.
  - ScalarE copies PSUM distance tiles to SBUF narrowed to bf16
    (round-to-nearest noise on the mins averages out over 32k rows/cols).
  - VectorE tensor_reduce(min) per 128-row tile -> row-mins; a bf16
    running accumulator updated with tensor_tensor(min) -> column-mins.
  - PE transposes the (128, 8192) column-min accumulator in 128x128
    blocks; VectorE segmented min-reduces produce per-column mins.
  - A final ones-matmul folds the partition axis (bf16 mins summed
    exactly in f32 PSUM) and a segmented add-reduce emits the two
    per-core scalars.

Host (~1 ms): transpose/scale views into the concat upload buffer; sum
the 8 fetched scalars in f64.
"""

import os
import numpy as np
import ml_dtypes

B = 4
N = 8192
M = 8192
CORES = 4                # data-parallel over B: one batch per core
P = 128                  # partitions
NT = N // P              # 64 n-tiles
CHUNK = 2048             # columns per PSUM macro-tile
NCH = M // CHUNK         # 4 chunks
MMF = 512                # matmul free dim (one PSUM bank of fp32)
KAUG = 9                 # augmented contraction size (fp32 rows)
NBLK = M // P            # 64 column blocks of 128 for the final fold
OUTW = NT + NBLK         # 128 output columns: [rowmin | colmin]

BF16 = ml_dtypes.bfloat16

_NC_CACHE = {}


def _build_nc():
    import concourse.bass as bass
    import concourse.mybir as mybir
    import concourse.tile as tile
    import concourse.bacc as bacc
    from concourse.masks import make_identity
    from contextlib import ExitStack

    f32 = mybir.dt.float32
    f32r = mybir.dt.float32r
    bf16 = mybir.dt.bfloat16
    MIN = mybir.AluOpType.min
    AXX = mybir.AxisListType.X
    SQUARE = mybir.ActivationFunctionType.Square

    ADD = mybir.AluOpType.add

    nc = bacc.Bacc(trn_type="TRN2")
    # single input tensor: rows 0:3 = a = -2*xyz1[b]^T, rows 3:6 = xyz2[b]^T.
    # float32r end-to-end (same f32 bytes on host): the BIR verifier requires
    # every producer feeding an fp32r matmul to emit fp32r-rounded output.
    pts_d = nc.dram_tensor("pts", (6, N), f32r, kind="ExternalInput").ap()
    # single tiny output: [sum of row-mins, sum of col-mins]
    out_d = nc.dram_tensor("out", (1, 2), f32, kind="ExternalOutput").ap()

    with tile.TileContext(nc) as tc, ExitStack() as ctx:
        consts = ctx.enter_context(tc.tile_pool(name="consts", bufs=1))
        accp = ctx.enter_context(tc.tile_pool(name="accp", bufs=1))
        psum = ctx.enter_context(tc.tile_pool(name="psum", bufs=2, space="PSUM"))
        dsb = ctx.enter_context(tc.tile_pool(name="dsb", bufs=3))
        outp = ctx.enter_context(tc.tile_pool(name="outp", bufs=1))

        # augmented matmul operands, built from the raw points via one
        # shared partition-0 scratch tile (see module docstring)
        aug1 = consts.tile([KAUG, N], f32r)
        aug2 = consts.tile([KAUG, M], f32r)
        scr = consts.tile([3, M], f32r)
        # memset/ACT don't codegen with float32r outputs, so they run on
        # plain-f32 bitcast views of the f32r tiles (byte-identical; only
        # the DMAs directly feeding the matmul need the f32r type)
        nc.sync.dma_start(out=aug1[0:3, :], in_=pts_d[0:3, :])
        nc.gpsimd.dma_start(out=aug2[0:3, :], in_=pts_d[3:6, :])
        nc.vector.memset(scr.bitcast(f32), 1.0)
        nc.sync.dma_start(out=aug1[6:9, :], in_=scr)
        nc.sync.dma_start(out=aug2[3:6, :], in_=scr)
        # p^2 rows: a = -2p, so (0.5*a)^2 = p^2
        nc.scalar.activation(out=scr.bitcast(f32), in_=aug1[0:3, :].bitcast(f32),
                             func=SQUARE, scale=0.5)
        nc.sync.dma_start(out=aug1[3:6, :], in_=scr)
        nc.scalar.activation(out=scr.bitcast(f32), in_=aug2[0:3, :].bitcast(f32),
                             func=SQUARE)
        nc.sync.dma_start(out=aug2[6:9, :], in_=scr)

        ident = consts.tile([P, P], bf16)
        make_identity(nc, ident)

        # column-min accumulator, bf16 (DVE tensor_tensor min runs at
        # 2x_1P for bf16 SBUF operands)
        acc = accp.tile([P, M], bf16)

        rmall = outp.tile([P, NT], f32)
        out_sb = outp.tile([P, OUTW], bf16)

        repeat = int(os.environ.get("CHAMFER_REPEAT", "1"))
        for rep in range(repeat):
          for t in range(NT):
            d = dsb.tile([P, M], bf16, tag="d")
            for c in range(NCH):
                ps = psum.tile([P, CHUNK], f32, tag="ps")
                for j in range(CHUNK // MMF):
                    col = c * CHUNK + j * MMF
                    # fp32r streams at 1 cyc/col (vs 4 for fp32) for moving
                    # dims >= 256; reduced multiply precision (~1e-5 on d,
                    # far under the bf16 narrowing noise)
                    nc.tensor.matmul(
                        ps[:, j * MMF:(j + 1) * MMF],
                        aug1[:, t * P:(t + 1) * P],
                        aug2[:, col:col + MMF],
                        start=True,
                        stop=True,
                    )
                # ACT copies + narrows to bf16 (min results only need bf16:
                # round-to-nearest noise averages out over 32k rows/cols)
                nc.scalar.copy(out=d[:, c * CHUNK:(c + 1) * CHUNK], in_=ps)

            nc.vector.tensor_reduce(
                out=rmall[:, t:t + 1], in_=d, axis=AXX, op=MIN
            )
            if t == 0 and rep == 0:
                nc.vector.tensor_copy(out=acc, in_=d)
            else:
                nc.vector.tensor_tensor(out=acc, in0=d, in1=acc, op=MIN)

        # row-min columns of the output (bf16 narrowing is lossless: the
        # f32 values are mins of bf16 numbers)
        nc.scalar.copy(out=out_sb[:, :NT], in_=rmall)

        # fold the column-min accumulator over the partition axis:
        # PE-transpose 128x128 bf16 blocks into PSUM, then segmented
        # min-reduce straight into the output tile.
        TGRP = 8
        for g in range(NBLK // TGRP):
            psT = psum.tile([P, TGRP * P], bf16, tag="ps")
            for j in range(TGRP):
                k = g * TGRP + j
                nc.tensor.transpose(
                    psT[:, j * P:(j + 1) * P], acc[:, k * P:(k + 1) * P], ident
                )
            seg = psT.rearrange("p (j x) -> p j x", x=P)
            nc.vector.tensor_reduce(
                out=out_sb[:, NT + g * TGRP:NT + (g + 1) * TGRP],
                in_=seg, axis=AXX, op=MIN,
            )

        # fold out_sb ([rowmin | colmin], (128, 128) bf16) to two scalars:
        # ones-matmul sums the partition axis into PSUM (exact: bf16 values
        # accumulated in f32), then a segmented add-reduce sums each half.
        ones128 = consts.tile([P, 1], bf16)
        nc.vector.memset(ones128, 1.0)
        psS = psum.tile([1, OUTW], f32, tag="ps")
        nc.tensor.matmul(psS, ones128, out_sb, start=True, stop=True)
        segS = psS.rearrange("p (j x) -> p j x", x=NT)
        out_fin = outp.tile([1, 2], f32)
        nc.vector.tensor_reduce(out=out_fin, in_=segS, axis=AXX, op=ADD)
        nc.sync.dma_start(out=out_d, in_=out_fin)
    nc.compile()
    return nc


def _get_runner():
    """Build (once) a cached jitted SPMD executor for the bass program.

    Mirrors concourse.bass2jax.run_bass_via_pjrt's multi-core path, but
    caches the jitted callable so repeat kernel() calls skip
    retrace/recompile, and fetches the single global output with one
    np.asarray call -- upload, execute and fetch then pipeline into a
    single tunnel round trip.
    """
    if "runner" in _NC_CACHE:
        return _NC_CACHE["runner"]

    import jax
    import concourse.mybir as mybir
    from jax.experimental.shard_map import shard_map
    from jax.sharding import Mesh, PartitionSpec
    from concourse.bass2jax import (
        install_neuronx_cc_hook,
        partition_id_tensor,
        _bass_exec_p,
    )

    install_neuronx_cc_hook()
    nc = _build_nc()

    in_names, out_names, out_avals, zero_outs = [], [], [], []
    partition_name = nc.partition_id_tensor.name if nc.partition_id_tensor else None
    for alloc in nc.m.functions[0].allocations:
        if not isinstance(alloc, mybir.MemoryLocationSet):
            continue
        name = alloc.memorylocations[0].name
        if alloc.kind == "ExternalInput":
            if name != partition_name:
                in_names.append(name)
        elif alloc.kind == "ExternalOutput":
            shape = tuple(alloc.tensor_shape)
            dtype = mybir.dt.np(alloc.dtype)
            out_names.append(name)
            out_avals.append(jax.core.ShapedArray(shape, dtype))
            zero_outs.append(np.zeros((CORES * shape[0], *shape[1:]), dtype))
    n_params = len(in_names)
    all_in_names = list(in_names) + list(out_names)
    if partition_name is not None:
        all_in_names.append(partition_name)
    donate = tuple(range(n_params, n_params + len(out_names)))

    def _body(*args):
        operands = list(args)
        if partition_name is not None:
            operands.append(partition_id_tensor())
        outs = _bass_exec_p.bind(
            *operands,
            out_avals=tuple(out_avals),
            in_names=tuple(all_in_names),
            out_names=tuple(out_names),
            lowering_input_output_aliases=(),
            sim_require_finite=True,
            sim_require_nnan=True,
            nc=nc,
        )
        return tuple(outs)

    devices = jax.devices()[:CORES]
    mesh = Mesh(np.asarray(devices), ("core",))
    in_specs = (PartitionSpec("core"),) * (n_params + len(out_names))
    out_specs = (PartitionSpec("core"),) * len(out_names)
    sharded = jax.jit(
        shard_map(
            _body, mesh=mesh, in_specs=in_specs, out_specs=out_specs, check_rep=False
        ),
        donate_argnums=donate,
        keep_unused=True,
    )

    def run(pts_cat):
        out = sharded(pts_cat, *zero_outs)
        # single fetch of the single global output: one round trip total
        return np.asarray(out[0])

    # the first couple of executions of a fresh jitted callable are ~2x
    # slower (dispatch fast-path + donation warm-up); absorb them into the
    # cold path so every caller-visible call runs at steady state
    dummy = np.zeros((CORES * 6, N), np.float32)
    for _ in range(2):
        run(dummy)

    _NC_CACHE["runner"] = run
    return run


def kernel(xyz1, xyz2):
    xyz1 = np.asarray(xyz1)
    xyz2 = np.asarray(xyz2)

    # per-core (= per-batch) input, concatenated along axis 0 for shard_map:
    # rows 0:3 = -2*xyz1[b]^T, rows 3:6 = xyz2[b]^T
    pts_cat = np.empty((CORES * 6, N), np.float32)
    for b in range(CORES):
        np.multiply(xyz1[b].T, -2.0, out=pts_cat[b * 6:b * 6 + 3])
        pts_cat[b * 6 + 3:b * 6 + 6] = xyz2[b].T

    out = _get_runner()(pts_cat)  # (CORES*1, 2) f32

    out64 = out.astype(np.float64)
    val = out64[:, 0].sum() / (B * N) + out64[:, 1].sum() / (B * M)
    return np.asarray(val, dtype=np.float32)


# revision 28
# speedup vs baseline: 1.0529x; 1.0237x over previous
"""Chamfer distance (squared-L2) kernel for Trainium2 NeuronCores (axon).

Problem: xyz1 (4, 8192, 3) f32, xyz2 (4, 8192, 3) f32.
  d[b,n,m] = ||p_n - q_m||^2 ; out = mean_n(min_m d) + mean_m(min_n d)  (scalar f32)

The warm-path cost in this environment is one axon tunnel round trip
(~60-90 ms) plus transfer bytes (~10-25 ms/MB); on-device compute (~1.5 ms)
is invisible inside that window.  The design therefore minimizes round
trips and bytes:

  - ONE round trip: a single cached jitted shard_map call with numpy args
    and a SINGLE output array fetched by a single np.asarray -- upload,
    execute and fetch all pipeline into one tunnel round trip.
  - Minimal upload: data-parallel over B on 4 cores, one full batch per
    core, so every point is uploaded exactly once (786 KB total: per core
    a = -2*xyz1[b]^T and q = xyz2[b]^T, both (3, 8192) f32).  Splitting N
    further across 8 cores would duplicate xyz2 (+50% bytes) for ~0.8 ms
    of hidden device time -- a strictly worse trade here.
  - Minimal fetch: per-core column-mins are COMPLETE (the core sees all
    rows of its batch), and both reductions are finished on device down
    to two scalars per core ([sum row-mins, sum col-mins], (1,2) f32) --
    the fetch is 32 bytes total.

Device kernel (per core, batch b):
  - Augmented K=9 fp32 matmul emits complete squared-distance tiles:
      lhsT = [a_x a_y a_z | p_x^2 p_y^2 p_z^2 | 1 1 1]      (9, 8192)
      rhs  = [q_x q_y q_z | 1     1     1     | q_x^2 q_y^2 q_z^2]
    so lhsT.T @ rhs = ||p||^2 + ||q||^2 - 2 p.q exactly in fp32.  The
    operand rows are built on device: squares via one ACT Square per side
    (scale=0.5 turns a=-2p into p^2), ones via memset; compute engines
    need 32-aligned SBUF partition bases, so these go through a
    partition-0 scratch tile and are DMA'd (no alignment constraint) into
    rows 3..8.  fp32 matmul streams at 4 cyc/col -> ~1.5 ms PE time.
  - ScalarE copies PSUM distance tiles to SBUF narrowed to bf16
    (round-to-nearest noise on the mins averages out over 32k rows/cols).
  - VectorE tensor_reduce(min) per 128-row tile -> row-mins; a bf16
    running accumulator updated with tensor_tensor(min) -> column-mins.
  - PE transposes the (128, 8192) column-min accumulator in 128x128
    blocks; VectorE segmented min-reduces produce per-column mins.
  - A final ones-matmul folds the partition axis (bf16 mins summed
    exactly in f32 PSUM) and a segmented add-reduce emits the two
    per-core scalars.

Host (~1 ms): transpose/scale views into the concat upload buffer; sum
the 8 fetched scalars in f64.
"""

import os
import numpy as np
import ml_dtypes

B = 4
N = 8192
M = 8192
CORES = 4                # data-parallel over B: one batch per core
P = 128                  # partitions
NT = N // P              # 64 n-tiles
CHUNK = 2048             # columns per PSUM macro-tile
NCH = M // CHUNK         # 4 chunks
MMF = 512                # matmul free dim (one PSUM bank of fp32)
KAUG = 9                 # augmented contraction size (fp32 rows)
NBLK = M // P            # 64 column blocks of 128 for the final fold
OUTW = NT + NBLK         # 128 output columns: [rowmin | colmin]

BF16 = ml_dtypes.bfloat16

_NC_CACHE = {}


def _build_nc():
    import concourse.bass as bass
    import concourse.mybir as mybir
    import concourse.tile as tile
    import concourse.bacc as bacc
    from concourse.masks import make_identity
    from contextlib import ExitStack

    f32 = mybir.dt.float32
    f32r = mybir.dt.float32r
    bf16 = mybir.dt.bfloat16
    MIN = mybir.AluOpType.min
    AXX = mybir.AxisListType.X
    SQUARE = mybir.ActivationFunctionType.Square

    ADD = mybir.AluOpType.add

    nc = bacc.Bacc(trn_type="TRN2")
    # single input tensor: rows 0:3 = a = -2*xyz1[b]^T, rows 3:6 = xyz2[b]^T.
    # float32r end-to-end (same f32 bytes on host): the BIR verifier requires
    # every producer feeding an fp32r matmul to emit fp32r-rounded output.
    pts_d = nc.dram_tensor("pts", (6, N), f32r, kind="ExternalInput").ap()
    # single tiny output: [sum of row-mins, sum of col-mins]
    out_d = nc.dram_tensor("out", (1, 2), f32, kind="ExternalOutput").ap()

    with tile.TileContext(nc) as tc, ExitStack() as ctx:
        consts = ctx.enter_context(tc.tile_pool(name="consts", bufs=1))
        accp = ctx.enter_context(tc.tile_pool(name="accp", bufs=1))
        psum = ctx.enter_context(tc.tile_pool(name="psum", bufs=2, space="PSUM"))
        dsb = ctx.enter_context(tc.tile_pool(name="dsb", bufs=3))
        outp = ctx.enter_context(tc.tile_pool(name="outp", bufs=1))

        # augmented matmul operands, built from the raw points via one
        # shared partition-0 scratch tile (see module docstring)
        aug1 = consts.tile([KAUG, N], f32r)
        aug2 = consts.tile([KAUG, M], f32r)
        scr = consts.tile([3, M], f32r)
        # memset/ACT don't codegen with float32r outputs, so they run on
        # plain-f32 bitcast views of the f32r tiles (byte-identical; only
        # the DMAs directly feeding the matmul need the f32r type)
        nc.sync.dma_start(out=aug1[0:3, :], in_=pts_d[0:3, :])
        nc.gpsimd.dma_start(out=aug2[0:3, :], in_=pts_d[3:6, :])
        nc.vector.memset(scr.bitcast(f32), 1.0)
        nc.sync.dma_start(out=aug1[6:9, :], in_=scr)
        nc.sync.dma_start(out=aug2[3:6, :], in_=scr)
        # p^2 rows: a = -2p, so (0.5*a)^2 = p^2
        nc.scalar.activation(out=scr.bitcast(f32), in_=aug1[0:3, :].bitcast(f32),
                             func=SQUARE, scale=0.5)
        nc.sync.dma_start(out=aug1[3:6, :], in_=scr)
        nc.scalar.activation(out=scr.bitcast(f32), in_=aug2[0:3, :].bitcast(f32),
                             func=SQUARE)
        nc.sync.dma_start(out=aug2[6:9, :], in_=scr)

        ident = consts.tile([P, P], bf16)
        make_identity(nc, ident)

        # column-min accumulator, bf16 (DVE tensor_tensor min runs at
        # 2x_1P for bf16 SBUF operands)
        acc = accp.tile([P, M], bf16)

        rmall = outp.tile([P, NT], f32)
        out_sb = outp.tile([P, OUTW], bf16)

        repeat = int(os.environ.get("CHAMFER_REPEAT", "1"))
        for rep in range(repeat):
          for t in range(NT):
            d = dsb.tile([P, M], bf16, tag="d")
            for c in range(NCH):
                ps = psum.tile([P, CHUNK], f32, tag="ps")
                for j in range(CHUNK // MMF):
                    col = c * CHUNK + j * MMF
                    # fp32r streams at 1 cyc/col (vs 4 for fp32) for moving
                    # dims >= 256; reduced multiply precision (~1e-5 on d,
                    # far under the bf16 narrowing noise)
                    nc.tensor.matmul(
                        ps[:, j * MMF:(j + 1) * MMF],
                        aug1[:, t * P:(t + 1) * P],
                        aug2[:, col:col + MMF],
                        start=True,
                        stop=True,
                    )
                # ACT copies + narrows to bf16 (min results only need bf16:
                # round-to-nearest noise averages out over 32k rows/cols)
                nc.scalar.copy(out=d[:, c * CHUNK:(c + 1) * CHUNK], in_=ps)

            nc.vector.tensor_reduce(
                out=rmall[:, t:t + 1], in_=d, axis=AXX, op=MIN
            )
            if t == 0 and rep == 0:
                nc.vector.tensor_copy(out=acc, in_=d)
            else:
                nc.vector.tensor_tensor(out=acc, in0=d, in1=acc, op=MIN)

        # row-min columns of the output (bf16 narrowing is lossless: the
        # f32 values are mins of bf16 numbers)
        nc.scalar.copy(out=out_sb[:, :NT], in_=rmall)

        # fold the column-min accumulator over the partition axis:
        # PE-transpose 128x128 bf16 blocks into PSUM, then segmented
        # min-reduce straight into the output tile.
        TGRP = 8
        for g in range(NBLK // TGRP):
            psT = psum.tile([P, TGRP * P], bf16, tag="ps")
            for j in range(TGRP):
                k = g * TGRP + j
                nc.tensor.transpose(
                    psT[:, j * P:(j + 1) * P], acc[:, k * P:(k + 1) * P], ident
                )
            seg = psT.rearrange("p (j x) -> p j x", x=P)
            nc.vector.tensor_reduce(
                out=out_sb[:, NT + g * TGRP:NT + (g + 1) * TGRP],
                in_=seg, axis=AXX, op=MIN,
            )

        # fold out_sb ([rowmin | colmin], (128, 128) bf16) to two scalars:
        # ones-matmul sums the partition axis into PSUM (exact: bf16 values
        # accumulated in f32), then a segmented add-reduce sums each half.
        ones128 = consts.tile([P, 1], bf16)
        nc.vector.memset(ones128, 1.0)
        psS = psum.tile([1, OUTW], f32, tag="ps")
        nc.tensor.matmul(psS, ones128, out_sb, start=True, stop=True)
        segS = psS.rearrange("p (j x) -> p j x", x=NT)
        out_fin = outp.tile([1, 2], f32)
        nc.vector.tensor_reduce(out=out_fin, in_=segS, axis=AXX, op=ADD)
        nc.sync.dma_start(out=out_d, in_=out_fin)
    nc.compile()
    return nc


def _get_runner():
    """Build (once) a cached jitted SPMD executor for the bass program.

    Mirrors concourse.bass2jax.run_bass_via_pjrt's multi-core path, but
    caches the jitted callable so repeat kernel() calls skip
    retrace/recompile, and fetches the single global output with one
    np.asarray call -- upload, execute and fetch then pipeline into a
    single tunnel round trip.
    """
    if "runner" in _NC_CACHE:
        return _NC_CACHE["runner"]

    import jax
    import concourse.mybir as mybir
    from jax.experimental.shard_map import shard_map
    from jax.sharding import Mesh, PartitionSpec
    from concourse.bass2jax import (
        install_neuronx_cc_hook,
        partition_id_tensor,
        _bass_exec_p,
    )

    install_neuronx_cc_hook()
    nc = _build_nc()

    in_names, out_names, out_avals, zero_outs = [], [], [], []
    partition_name = nc.partition_id_tensor.name if nc.partition_id_tensor else None
    for alloc in nc.m.functions[0].allocations:
        if not isinstance(alloc, mybir.MemoryLocationSet):
            continue
        name = alloc.memorylocations[0].name
        if alloc.kind == "ExternalInput":
            if name != partition_name:
                in_names.append(name)
        elif alloc.kind == "ExternalOutput":
            shape = tuple(alloc.tensor_shape)
            dtype = mybir.dt.np(alloc.dtype)
            out_names.append(name)
            out_avals.append(jax.core.ShapedArray(shape, dtype))
            zero_outs.append(np.zeros((CORES * shape[0], *shape[1:]), dtype))
    n_params = len(in_names)
    all_in_names = list(in_names) + list(out_names)
    if partition_name is not None:
        all_in_names.append(partition_name)
    donate = tuple(range(n_params, n_params + len(out_names)))

    def _body(*args):
        operands = list(args)
        if partition_name is not None:
            operands.append(partition_id_tensor())
        outs = _bass_exec_p.bind(
            *operands,
            out_avals=tuple(out_avals),
            in_names=tuple(all_in_names),
            out_names=tuple(out_names),
            lowering_input_output_aliases=(),
            sim_require_finite=True,
            sim_require_nnan=True,
            nc=nc,
        )
        return tuple(outs)

    devices = jax.devices()[:CORES]
    mesh = Mesh(np.asarray(devices), ("core",))
    in_specs = (PartitionSpec("core"),) * (n_params + len(out_names))
    out_specs = (PartitionSpec("core"),) * len(out_names)
    sharded = jax.jit(
        shard_map(
            _body, mesh=mesh, in_specs=in_specs, out_specs=out_specs, check_rep=False
        ),
        donate_argnums=donate,
        keep_unused=True,
    )

    def run(pts_cat):
        out = sharded(pts_cat, *zero_outs)
        # single fetch of the single global output: one round trip total
        return np.asarray(out[0])

    # the first couple of executions of a fresh jitted callable are ~2x
    # slower (dispatch fast-path + donation warm-up); absorb them into the
    # cold path so every caller-visible call runs at steady state
    dummy = np.zeros((CORES * 6, N), np.float32)
    for _ in range(2):
        run(dummy)

    _NC_CACHE["runner"] = run
    return run


def kernel(xyz1, xyz2):
    xyz1 = np.asarray(xyz1)
    xyz2 = np.asarray(xyz2)

    # per-core (= per-batch) input, concatenated along axis 0 for shard_map:
    # rows 0:3 = -2*xyz1[b]^T, rows 3:6 = xyz2[b]^T
    pts_cat = np.empty((CORES * 6, N), np.float32)
    for b in range(CORES):
        np.multiply(xyz1[b].T, -2.0, out=pts_cat[b * 6:b * 6 + 3])
        pts_cat[b * 6 + 3:b * 6 + 6] = xyz2[b].T

    out = _get_runner()(pts_cat)  # (CORES*1, 2) f32

    out64 = out.astype(np.float64)
    val = out64[:, 0].sum() / (B * N) + out64[:, 1].sum() / (B * M)
    return np.asarray(val, dtype=np.float32)
